# revision 13
# baseline (speedup 1.0000x reference)
"""D4-pool Trainium2 kernel.

x: [256, 128, 64, 64] f32. Groups of 8 consecutive batch entries hold the 8
D4 orientations of one image; undo each orientation and mean over the group,
giving [32, 128, 64, 64].

Sharding: data-parallel over the group dim — core k gets groups [4k, 4k+4)
(batch entries [32k, 32k+32)), so the reduce is fully device-local.

Layout trick: with C (=128) on SBUF partitions and (H, W) on the free dim,
every D4 inverse transform is pure free-dim address arithmetic (stride ±1 /
±64 access patterns) — no transpose instructions, no partition movement.
Per partition, the required inverse-transform reads are:
  o=0: A[h, w]          o=1: A[w, 63-h]     o=2: A[63-h, 63-w]
  o=3: A[63-w, h]       o=4: A[h, 63-w]     o=5: A[w, h]
  o=6: A[63-h, w]       o=7: A[63-w, 63-h]
Loads/stores are fully contiguous 2 MiB DMAs; DVE does the accumulation
(1/8-scale folded in); ACT initializes accumulators off the critical path.
Measured ~200 us/core on hardware ≈ the HBM/DMA line-rate roofline
(75.5 MB/core through 16 SDMA engines at ~27 GiB/s each = ~181 us).
"""

import sys

for _p in ("/opt/trn_rl_repo",):
    if _p not in sys.path:
        sys.path.insert(0, _p)

import numpy as np

import concourse.bacc as bacc
import concourse.mybir as mybir
from concourse.bass_utils import run_bass_kernel_spmd
from concourse.tile import TileContext

N_CORES = 8
B, C, H, W = 256, 128, 64, 64
ENTRIES_PER_CORE = B // N_CORES          # 32 batch entries
GROUPS_PER_CORE = ENTRIES_PER_CORE // 8  # 4 groups of 8 orientations


def build_nc(groups: int = GROUPS_PER_CORE) -> bacc.Bacc:
    f32 = mybir.dt.float32
    nc = bacc.Bacc()
    x = nc.declare_dram_parameter("x", [groups * 8, C, H, W], f32, isOutput=False)
    y = nc.declare_dram_parameter("y", [groups, C, H, W], f32, isOutput=True)

    # Two accumulators per group so only ONE DVE op per group pays the
    # slow inner-stride-64 (transposed) read:
    #   acc  [c,h,w]: init = x0*1/8 (ACT), += o=2,4,6 (flip APs, stride ±1)
    #   accT [c,w,h]: init = x5*1/8 (ACT; pure transpose == contiguous),
    #                 += o=1,3,7 (flips in transposed coords, stride ±1)
    # The 1/8 scale folds into every accumulate (DVE STT: term*s + acc),
    # so nothing post-combine remains but the store. Combine + store run
    # in H-halves so the first half's store overlaps the second half.
    # accT-side APs: accT[w,h] += A1[w,63-h] / A3[63-w,h] / A7[63-w,63-h].
    accT_slice = {1: lambda t: t[:, :, ::-1], 3: lambda t: t[:, ::-1, :],
                  7: lambda t: t[:, ::-1, ::-1]}
    acc_slice = {2: lambda t: t[:, ::-1, ::-1], 4: lambda t: t[:, :, ::-1],
                 6: lambda t: t[:, ::-1, :]}
    mult, add = mybir.AluOpType.mult, mybir.AluOpType.add
    with TileContext(nc) as tc:
        with (
            tc.tile_pool(name="xin", bufs=8) as xin_pool,
            tc.tile_pool(name="acc", bufs=2) as acc_pool,
            tc.tile_pool(name="accT", bufs=2) as accT_pool,
        ):
            for g in range(groups):
                acc = acc_pool.tile([C, H, W], f32, tag="acc")
                accT = accT_pool.tile([C, H, W], f32, tag="accT")
                last = g == groups - 1
                for o in (0, 5, 1, 2, 3, 4, 6, 7):
                    xt = xin_pool.tile([C, H, W], f32, tag="xin")
                    nc.sync.dma_start(xt[:, :, :], x[8 * g + o])
                    if o == 0:
                        nc.scalar.mul(acc[:, :, :], xt[:, :, :], 0.125)
                    elif o == 5:
                        nc.scalar.mul(accT[:, :, :], xt[:, :, :], 0.125)
                    elif o == 7 and last:
                        # Tail of the whole kernel: process the final
                        # orientation, combine, and store in H-halves so
                        # the first half's store overlaps the second
                        # half's compute.
                        for h0 in (0, H // 2):
                            hs = slice(h0, h0 + H // 2)
                            nc.vector.scalar_tensor_tensor(
                                accT[:, :, hs], accT_slice[7](xt)[:, :, hs],
                                0.125, accT[:, :, hs], mult, add,
                            )
                            nc.vector.tensor_add(
                                acc[:, hs, :], acc[:, hs, :],
                                accT[:, :, hs].transpose([0, 2, 1]),
                            )
                            nc.scalar.dma_start(y[g][:, hs, :], acc[:, hs, :])
                    elif o in accT_slice:
                        nc.vector.scalar_tensor_tensor(
                            accT[:, :, :], accT_slice[o](xt), 0.125,
                            accT[:, :, :], mult, add,
                        )
                    else:
                        nc.vector.scalar_tensor_tensor(
                            acc[:, :, :], acc_slice[o](xt), 0.125,
                            acc[:, :, :], mult, add,
                        )
                if not last:
                    for h0 in (0, H // 2):
                        hs = slice(h0, h0 + H // 2)
                        nc.vector.tensor_add(
                            acc[:, hs, :], acc[:, hs, :],
                            accT[:, :, hs].transpose([0, 2, 1]),
                        )
                        # Store on the ACT HWDGE queue — keeps the
                        # compute-gated store from head-of-line blocking
                        # loads on sync's queue.
                        nc.scalar.dma_start(y[g][:, hs, :], acc[:, hs, :])
    nc.compile()
    return nc


_NC_CACHE: list = []


def run(x: np.ndarray, trace: bool = False, **spmd_kwargs):
    """Shard, run on all 8 cores, gather. Returns (output, BassKernelResults)."""
    x = np.ascontiguousarray(x, dtype=np.float32)
    assert x.shape == (B, C, H, W), x.shape
    shards = x.reshape(N_CORES, ENTRIES_PER_CORE, C, H, W)
    if not _NC_CACHE:
        _NC_CACHE.append(build_nc())
    nc = _NC_CACHE[0]
    in_maps = [{"x": shards[i]} for i in range(N_CORES)]
    res = run_bass_kernel_spmd(
        nc, in_maps, list(range(N_CORES)), trace=trace, **spmd_kwargs
    )
    out = np.concatenate([res.results[i]["y"] for i in range(N_CORES)], axis=0)
    return out, res


def kernel(x: np.ndarray) -> np.ndarray:
    out, _ = run(x)
    return out
